# revision 1
# baseline (speedup 1.0000x reference)
"""
Multi-head attention Trainium2 Bass kernel (B=16, S=1024, D=768, H=12, Dh=64).

Sharding: data parallel over batch — 8 cores x 2 batches each. Weights are
replicated; no collectives.

Per-core device algorithm (all matmuls bf16 with fp32 PSUM accumulation):
  1. QK^T projection: per head-pair tiles [Q^T_h0; Q^T_h1] and [K^T_h0; K^T_h1]
     of shape [128, S] (partition = head-dim e, stacked 2 heads), computed as
     lhsT = [W_h0 | W_h1] (stationary), rhs = X^T.  bq added on the PSUM->SBUF
     copy (per-partition scalar); bk is skipped entirely (constant-per-row
     terms cancel in softmax).
  2. V projection in [t, e] layout with a zero column per head that is later
     memset to 1 (V' = [V_h | 1]) -> AV matmul also produces softmax row-sums.
  3. scores^T tiles [t, s] via row-tiled (tile_position) pairs of K=64 matmuls
     (2 heads concurrently in the 128x128 array).  Q is pre-scaled by 1/256 so
     the scores PSUM holds u = raw/256; softmax runs without max subtraction
     (u in ~[-0.2, 0.2], exp(32u) is safe in fp32): ACT exp (scale=32) fused
     with the PSUM->SBUF copy.  Optionally a fraction of the exp tiles can be
     routed to a custom 2-op DVE exp (dve_k > 0).
  4. AV: O'^T[e|rowsum, s] = V'^T A^T accumulated over t tiles.
  5. normalize: recip(rowsum) -> gpsimd partition-broadcast -> multiply ->
     msa^T.
  6. out-projection Y^T = Wo^T msa^T + bo' where bo' = bo + bv_flat @ Wo
     (folded on host), written to DRAM as Y^T and transposed on host.

Scheduling: the two per-core batches are pipelined — the next batch's QKV
projection matmuls (and the previous batch's output projection) are
interleaved into the attention microloop in ~2-matmul units, so the tensor
engine fills the gaps of the ACT(exp)-gated attention phase.
"""

import sys

for p in ("/opt/trn_rl_repo", "/root/.axon_site/_ro/trn_rl_repo"):
    if p not in sys.path:
        sys.path.insert(0, p)

import numpy as np
import ml_dtypes

B, S, D, H, Dh = 16, 1024, 768, 12, 64
NCORE = 8
BLOC = B // NCORE          # 2 batches per core
PAIRS = H // 2             # 6 head pairs
DT = D // 128              # 6 d-tiles (contraction tiles)
TT = S // 128              # 8 t-tiles
SC = S // 512              # 2 s-chunks
VW = H * (Dh + 1)          # 780: V' width incl. ones columns

_CACHE = {}

# ---- custom DVE exp (softmax exp offload from ACT to DVE) ------------------
# exp(32*u) = (1 + u + u^2/2 + u^3/6)^32 for |u| <= ~0.2 (scores pre-scaled
# by 1/256 so PSUM holds u).  Two DVE ops: cubic Horner, then 5 squarings.
_EXPA_CONSTS = {"s0": 1.0 / 6.0, "s1": 0.5, "imm2": 1.0}
_DVE_OPS_CACHE = {}


def _get_exp_ops():
    if "ops" in _DVE_OPS_CACHE:
        return _DVE_OPS_CACHE["ops"]
    import numpy as _np
    from concourse.dve_spec import Spec, Src0, C0, C1, C2, sq, lower, _has_src1
    from concourse.dve_uop import DveOpSpec
    from concourse.dve_ops import (
        DveOp, OPS, _SUB_OPCODE_FOR_NAME, CUSTOM_DVE_SPECS)

    def make_op(name, spec, subdim=False):
        if name not in _SUB_OPCODE_FOR_NAME:
            _SUB_OPCODE_FOR_NAME[name] = 1 + len(OPS)
        shas = {}
        for ver in ("v3", "v4"):
            uops = lower(spec, ver=ver)
            shas[ver] = DveOpSpec(
                name=name, opcode=_SUB_OPCODE_FOR_NAME[name],
                uops=uops, rd1_en=_has_src1(spec)).sha(ver)
        op = DveOp(name, spec, subdim=subdim, uops_sha=shas)
        if all(o.name != name for o in OPS):
            OPS.append(op)
        CUSTOM_DVE_SPECS[name] = spec
        return op

    poly = make_op(
        "EXP32_POLY_ANT",
        Spec(body=((Src0 * C0 + C1) * Src0 + C2) * Src0 + C2,
             reference=lambda in0, in1, c0, c1, c2:
             ((in0 * c0 + c1) * in0 + c2) * in0 + c2))
    sq5 = make_op(
        "EXP32_SQ5_ANT",
        Spec(body=sq(sq(sq(sq(sq(Src0))))),
             reference=lambda in0, in1, c0, c1, c2:
             ((((in0 * in0) ** 2) ** 2) ** 2) ** 2))
    _DVE_OPS_CACHE["ops"] = (poly, sq5)
    return poly, sq5


def _emit_exp32(nc, out_bf, w_f32, psum_in):
    poly, sq5 = _get_exp_ops()
    c = _EXPA_CONSTS
    nc.vector._custom_dve(poly, out=w_f32, in0=psum_in,
                          s0=c["s0"], s1=c["s1"], imm2=c["imm2"])
    nc.vector._custom_dve(sq5, out=out_bf, in0=w_f32)



def _build_program(repeats=1, ablate=None, dve_k=0):
    import concourse.tile as tile
    from concourse import bacc, mybir

    bf = mybir.dt.bfloat16
    f32 = mybir.dt.float32
    EXP = mybir.ActivationFunctionType.Exp

    nc = bacc.Bacc("TRN2", target_bir_lowering=False, debug=False,
                   num_devices=NCORE)

    XT = nc.dram_tensor("XT", [BLOC, D, S], bf, kind="ExternalInput").ap()
    WQ = nc.dram_tensor("WQ", [D, D], bf, kind="ExternalInput").ap()
    WK = nc.dram_tensor("WK", [D, D], bf, kind="ExternalInput").ap()
    WV = nc.dram_tensor("WV", [D, VW], bf, kind="ExternalInput").ap()
    WO = nc.dram_tensor("WO", [D, D], bf, kind="ExternalInput").ap()
    BQ = nc.dram_tensor("BQ", [128, PAIRS], f32, kind="ExternalInput").ap()
    BO = nc.dram_tensor("BO", [128, DT], f32, kind="ExternalInput").ap()
    YT = nc.dram_tensor("YT", [BLOC, D, S], f32, kind="ExternalOutput").ap()

    VB = BLOC * repeats  # virtual batches (b = vb % BLOC)
    # which t-tiles of each (pair, chunk) run softmax-exp on DVE instead of ACT
    dve_ts = {0: (), 1: (4,), 2: (2, 6), 3: (1, 4, 6),
              4: (1, 3, 5, 7)}[dve_k]

    with tile.TileContext(nc) as tc:
        import contextlib
        with contextlib.ExitStack() as ctx:
            consts = ctx.enter_context(tc.tile_pool(name="consts", bufs=1))
            xt_p = ctx.enter_context(tc.tile_pool(name="xt", bufs=2 * DT))
            qk_p = ctx.enter_context(tc.tile_pool(name="qk", bufs=4 * PAIRS))
            vp_p = ctx.enter_context(tc.tile_pool(name="vp", bufs=2 * TT))
            a_p = ctx.enter_context(tc.tile_pool(name="a", bufs=4))
            aw_p = ctx.enter_context(tc.tile_pool(name="aw", bufs=2))
            msa_p = ctx.enter_context(tc.tile_pool(name="msa", bufs=2 * DT))
            y_p = ctx.enter_context(tc.tile_pool(name="y", bufs=4))
            r_p = ctx.enter_context(tc.tile_pool(name="r", bufs=4))
            rb_p = ctx.enter_context(tc.tile_pool(name="rb", bufs=4))
            # PSUM (8 banks): scores 2x[128,1024]=4, qkv/v/proj 1x[128,1024]=2,
            # AV accumulators 2x[65,512]=2
            ps_sc = ctx.enter_context(
                tc.tile_pool(name="ps_sc", bufs=2, space="PSUM"))
            ps_mm = ctx.enter_context(
                tc.tile_pool(name="ps_mm", bufs=1, space="PSUM"))
            ps_av = ctx.enter_context(
                tc.tile_pool(name="ps_av", bufs=2, space="PSUM"))

            # ---- resident weights / biases -------------------------------
            wq_sb = []
            wk_sb = []
            wv_sb = []
            wo_sb = []
            for d in range(DT):
                t = consts.tile([128, D], bf, tag=f"wq{d}")
                nc.sync.dma_start(out=t, in_=WQ[d * 128:(d + 1) * 128, :])
                wq_sb.append(t)
                t = consts.tile([128, D], bf, tag=f"wk{d}")
                nc.sync.dma_start(out=t, in_=WK[d * 128:(d + 1) * 128, :])
                wk_sb.append(t)
                t = consts.tile([128, VW], bf, tag=f"wv{d}")
                nc.sync.dma_start(out=t, in_=WV[d * 128:(d + 1) * 128, :])
                wv_sb.append(t)
                t = consts.tile([128, D], bf, tag=f"wo{d}")
                nc.sync.dma_start(out=t, in_=WO[d * 128:(d + 1) * 128, :])
                wo_sb.append(t)
            bq_sb = consts.tile([128, PAIRS], f32, tag="bq")
            nc.sync.dma_start(out=bq_sb, in_=BQ)
            bo_sb = consts.tile([128, DT], f32, tag="bo")
            nc.sync.dma_start(out=bo_sb, in_=BO)

            def load_xt(vb):
                b = vb % BLOC
                xs = []
                for d in range(DT):
                    t = xt_p.tile([128, S], bf, tag="xt", name=f"xt{d}")
                    nc.sync.dma_start(
                        out=t, in_=XT[b, d * 128:(d + 1) * 128, :])
                    xs.append(t)
                return xs

            def qkv_stream(xt_sb, qt_sb, kt_sb, vp_sb, pools=None):
                """Generator: emits the QKV projections in ~2-matmul units;
                appends finished tiles to the given lists.  During the dense
                startup (attention idle) the scores PSUM pool is also free, so
                `pools` can alternate groups across both pools to double-buffer
                the group->copy chain."""
                pools = pools or [(ps_mm, "ps_mm")]
                gi = 0
                for p in range(PAIRS):
                    for kind in ("q", "k"):
                        w = wq_sb if kind == "q" else wk_sb
                        pool, ptag = pools[gi % len(pools)]
                        gi += 1
                        ps = pool.tile([128, S], f32, tag=ptag,
                                       name="ps_qk")
                        for d in range(DT):
                            for c in range(SC):
                                nc.tensor.matmul(
                                    ps[:, c * 512:(c + 1) * 512],
                                    lhsT=w[d][:, p * 128:(p + 1) * 128],
                                    rhs=xt_sb[d][:, c * 512:(c + 1) * 512],
                                    start=(d == 0), stop=(d == DT - 1))
                            yield
                        out = qk_p.tile([128, S], bf, tag="qk", name="qk")
                        if kind == "q":
                            # Q scaled by 1/256 so scores PSUM holds u =
                            # raw/256 (for exp(32u) on both ACT and DVE paths)
                            nc.vector.tensor_scalar(
                                out, ps, 1.0 / 256.0, bq_sb[:, p:p + 1],
                                mybir.AluOpType.mult, mybir.AluOpType.add)
                            qt_sb.append(out)
                        else:
                            nc.vector.tensor_copy(out, ps)
                            kt_sb.append(out)
                        yield
                for T in range(TT):
                    pool, ptag = pools[gi % len(pools)]
                    gi += 1
                    ps = pool.tile([128, S], f32, tag=ptag, name="ps_v")
                    for d in range(DT):
                        nc.tensor.matmul(
                            ps[:, 0:512],
                            lhsT=xt_sb[d][:, T * 128:(T + 1) * 128],
                            rhs=wv_sb[d][:, 0:512],
                            start=(d == 0), stop=(d == DT - 1))
                        nc.tensor.matmul(
                            ps[:, 512:VW],
                            lhsT=xt_sb[d][:, T * 128:(T + 1) * 128],
                            rhs=wv_sb[d][:, 512:VW],
                            start=(d == 0), stop=(d == DT - 1))
                        yield
                    vp = vp_p.tile([128, VW], bf, tag="vp", name="vp")
                    nc.vector.tensor_copy(vp, ps[:, 0:VW])
                    v3 = vp.rearrange("p (h e) -> p h e", e=Dh + 1)
                    nc.vector.memset(v3[:, :, Dh:Dh + 1], 1.0)
                    vp_sb.append(vp)
                    yield

            def proj_stream(b, msa_sb, pools=None):
                """Generator: output projection + bias + DMA out.  The dense
                tail can alternate PSUM pools (scores pool is idle then)."""
                pools = pools or [(ps_mm, "ps_mm")]
                for o in range(DT):
                    pool, ptag = pools[o % len(pools)]
                    ps = pool.tile([128, S], f32, tag=ptag, name="ps_o")
                    for d in range(DT):
                        for c in range(SC):
                            nc.tensor.matmul(
                                ps[:, c * 512:(c + 1) * 512],
                                lhsT=wo_sb[d][:, o * 128:(o + 1) * 128],
                                rhs=msa_sb[d][:, c * 512:(c + 1) * 512],
                                start=(d == 0), stop=(d == DT - 1))
                        yield
                    y = y_p.tile([128, S], f32, tag="y", name="y")
                    nc.vector.tensor_scalar_add(y, ps, bo_sb[:, o:o + 1])
                    if ablate != "no_out_dma":
                        nc.sync.dma_start(
                            out=YT[b, o * 128:(o + 1) * 128, :], in_=y)
                    yield

            _done = object()

            def pull(gen, n):
                if gen is not None:
                    for _ in range(n):
                        if next(gen, _done) is _done:
                            break

            def attention(qt_sb, kt_sb, vp_sb, msa_sb, foreign=None):
                for p in range(PAIRS):
                    msa = msa_p.tile([128, S], bf, tag="msa", name="msa")
                    msa_sb.append(msa)
                    for c in range(SC):
                        po = [ps_av.tile([65, 512], f32, tag="ps_av",
                                         name=f"po{h}")
                              for h in range(2)]

                        def emit_av(T, at):
                            for h in range(2):
                                nc.tensor.matmul(
                                    po[h],
                                    lhsT=vp_sb[T][
                                        :, (2 * p + h) * (Dh + 1):
                                        (2 * p + h + 1) * (Dh + 1)],
                                    rhs=at[:, h * 512:(h + 1) * 512],
                                    start=(T == 0), stop=(T == TT - 1))

                        # AV runs one t-tile behind exp so the PE queue never
                        # blocks on ACT: FIFO is scores(T) -> foreign ->
                        # AV(T-1), with exp(T) on ACT in parallel.
                        pend = None
                        for T in range(TT):
                            pss = ps_sc.tile([128, 1024], f32, tag="ps_sc",
                                             name="pss")
                            nh = 1 if ablate == "half_scores" else 2
                            for h in range(nh):
                                nc.tensor.matmul(
                                    pss[:, h * 512:(h + 1) * 512],
                                    lhsT=kt_sb[p][h * 64:(h + 1) * 64,
                                                  T * 128:(T + 1) * 128],
                                    rhs=qt_sb[p][h * 64:(h + 1) * 64,
                                                 c * 512:(c + 1) * 512],
                                    start=True, stop=True,
                                    tile_position=(
                                        None if ablate == "no_rowtile"
                                        else (h * 64, 0)))
                            at = a_p.tile([128, 1024], bf, tag="a", name="at")
                            if ablate == "half_exp":
                                nc.scalar.activation(
                                    at[:, 0:512], pss[:, 0:512], EXP,
                                    scale=32.0)
                            elif T in dve_ts:
                                aw = aw_p.tile([128, 1024], f32, tag="aw",
                                               name="aw")
                                _emit_exp32(nc, at, aw, pss)
                            else:
                                nc.scalar.activation(at, pss, EXP, scale=32.0)
                            pull(foreign, 2)
                            if pend is not None:
                                emit_av(*pend)
                            pend = (T, at)
                        emit_av(*pend)
                        for h in range(2):
                            r = r_p.tile([1, 512], f32, tag="r", name="r")
                            nc.vector.reciprocal(r, po[h][64:65, :])
                            rb = rb_p.tile([64, 512], f32, tag="rb", name="rb")
                            nc.gpsimd.partition_broadcast(rb, r)
                            nc.vector.tensor_mul(
                                msa[h * 64:(h + 1) * 64,
                                    c * 512:(c + 1) * 512],
                                po[h][0:64, :], rb)
                        pull(foreign, 2)

            # ---- pipelined schedule over virtual batches ------------------
            qts = {}
            kts = {}
            vps = {}
            msas = {}
            xts = {}
            streams = {}

            xts[0] = load_xt(0)
            # ACT exp-table load (~2.7us) off the critical path: a dummy exp
            # during the QKV phase triggers PSEUDO_LOAD_ACT_FUNC_SET early.
            warm = consts.tile([1, 2], f32, tag="warm")
            nc.vector.memset(warm, 0.0)
            nc.scalar.activation(warm, warm, EXP)
            streams[0] = qkv_stream(xts[0], qts.setdefault(0, []),
                                    kts.setdefault(0, []),
                                    vps.setdefault(0, []),
                                    pools=[(ps_mm, "ps_mm"),
                                           (ps_sc, "ps_sc")])
            pull(streams[0], 10**9)  # dense startup
            for vb in range(VB):
                gens = []
                if vb > 0:
                    gens.append(proj_stream((vb - 1) % BLOC, msas[vb - 1]))
                if vb + 1 < VB:
                    xts[vb + 1] = load_xt(vb + 1)
                    streams[vb + 1] = qkv_stream(
                        xts[vb + 1], qts.setdefault(vb + 1, []),
                        kts.setdefault(vb + 1, []), vps.setdefault(vb + 1, []))
                    gens.append(streams[vb + 1])
                import itertools
                foreign = itertools.chain(*gens) if gens else None
                msas[vb] = []
                attention(qts[vb], kts[vb], vps[vb], msas[vb], foreign=foreign)
                pull(foreign, 10**9)  # drain leftovers
                # free references to recycled tiles
                for dd in (qts, kts, vps, xts):
                    dd.pop(vb - 1, None)
            # dense tail: last projection (scores pool idle -> alternate)
            pull(proj_stream((VB - 1) % BLOC, msas[VB - 1],
                             pools=[(ps_mm, "ps_mm"), (ps_sc, "ps_sc")]),
                 10**9)

    nc.compile()
    return nc


def _prep_inputs(X, Wq, bq, Wk, bk, Wv, bv, Wo, bo):
    bf16 = ml_dtypes.bfloat16
    X = np.asarray(X, dtype=np.float32)
    # per-core X^T: [core][BLOC, D, S]
    xt = np.ascontiguousarray(
        X.reshape(NCORE, BLOC, S, D).transpose(0, 1, 3, 2)).astype(bf16)
    wq = np.ascontiguousarray(
        np.asarray(Wq, np.float32).transpose(1, 0, 2).reshape(D, D)).astype(bf16)
    wk = np.ascontiguousarray(
        np.asarray(Wk, np.float32).transpose(1, 0, 2).reshape(D, D)).astype(bf16)
    wv = np.zeros((D, VW), np.float32)
    Wv = np.asarray(Wv, np.float32)
    for h in range(H):
        wv[:, h * (Dh + 1):h * (Dh + 1) + Dh] = Wv[h]
    wv = wv.astype(bf16)
    wo = np.asarray(Wo, np.float32).astype(bf16)
    bq2 = np.ascontiguousarray(
        np.asarray(bq, np.float32).reshape(PAIRS, 128).T) / 256.0
    bo_eff = np.asarray(bo, np.float32) + \
        np.asarray(bv, np.float32).reshape(-1) @ np.asarray(Wo, np.float32)
    bo2 = np.ascontiguousarray(bo_eff.reshape(DT, 128).T.astype(np.float32))
    in_maps = [
        {"XT": xt[c], "WQ": wq, "WK": wk, "WV": wv, "WO": wo,
         "BQ": bq2, "BO": bo2}
        for c in range(NCORE)
    ]
    return in_maps


def _get_runner(repeats=1, ablate=None, dve_k=0):
    """Build (once) a jitted SPMD runner over the 8 cores, modeled on
    bass2jax.run_bass_via_pjrt but cached so repeat calls don't re-trace."""
    key = ("runner", repeats, ablate, dve_k)
    if key in _CACHE:
        return _CACHE[key]

    import jax
    import numpy as _np
    from jax.sharding import Mesh, PartitionSpec, NamedSharding
    from jax.experimental.shard_map import shard_map
    from concourse import mybir
    from concourse.bass2jax import (
        _bass_exec_p, install_neuronx_cc_hook, partition_id_tensor)

    nc = _build_program(repeats=repeats, ablate=ablate, dve_k=dve_k)
    install_neuronx_cc_hook()

    import concourse.mybir as _mybir
    in_names, out_names, out_avals, zero_shapes = [], [], [], []
    partition_name = (nc.partition_id_tensor.name
                      if nc.partition_id_tensor else None)
    for alloc in nc.m.functions[0].allocations:
        if not isinstance(alloc, _mybir.MemoryLocationSet):
            continue
        name = alloc.memorylocations[0].name
        if alloc.kind == "ExternalInput":
            if name != partition_name:
                in_names.append(name)
        elif alloc.kind == "ExternalOutput":
            shape = tuple(alloc.tensor_shape)
            dtype = _mybir.dt.np(alloc.dtype)
            out_names.append(name)
            out_avals.append(jax.core.ShapedArray(shape, dtype))
            zero_shapes.append((shape, dtype))
    n_params = len(in_names)
    n_outs = len(out_names)
    all_in_names = in_names + out_names
    if partition_name is not None:
        all_in_names = all_in_names + [partition_name]

    def _body(*args):
        operands = list(args)
        if partition_name is not None:
            operands.append(partition_id_tensor())
        outs = _bass_exec_p.bind(
            *operands,
            out_avals=tuple(out_avals),
            in_names=tuple(all_in_names),
            out_names=tuple(out_names),
            lowering_input_output_aliases=(),
            sim_require_finite=True,
            sim_require_nnan=True,
            nc=nc,
        )
        return tuple(outs)

    devices = jax.devices()[:NCORE]
    mesh = Mesh(_np.asarray(devices), ("core",))
    in_specs = (PartitionSpec("core"),) * (n_params + n_outs)
    out_specs = (PartitionSpec("core"),) * n_outs
    # NOTE: no donation — the kernel writes every output element, so the
    # custom call's self-allocated (uninit) output buffers are fine, and the
    # zero "output operand" arrays can be created once and reused across
    # calls instead of being shipped host->device (50 MB) per call.
    sharded = jax.jit(
        shard_map(_body, mesh=mesh, in_specs=in_specs, out_specs=out_specs,
                  check_rep=False),
        keep_unused=True)
    shard = NamedSharding(mesh, PartitionSpec("core"))
    import jax.numpy as jnp
    zeros_dev = [
        jax.device_put(_np.zeros((NCORE * s[0], *s[1:]), d), shard)
        for s, d in zero_shapes
    ]

    def put_inputs(in_maps):
        # concatenate along axis 0 (per-core stacking)
        concat = []
        for nm in in_names:
            arrs = [_np.asarray(in_maps[c][nm]) for c in range(NCORE)]
            concat.append(_np.concatenate(arrs, axis=0))
        return [jax.device_put(a, shard) for a in concat]

    _CACHE[("sharded", repeats, ablate, dve_k)] = (sharded, zeros_dev)

    def run(dev_inputs):
        outs = sharded(*dev_inputs, *zeros_dev)
        jax.block_until_ready(outs)
        return outs

    def unpack(outs):
        res = []
        for c in range(NCORE):
            d = {}
            for i, nm in enumerate(out_names):
                full = _np.asarray(outs[i])
                d[nm] = full.reshape(NCORE, *out_avals[i].shape)[c]
            res.append(d)
        return res

    _CACHE[key] = (put_inputs, run, unpack)
    return _CACHE[key]


def kernel(X, Wq, bq, Wk, bk, Wv, bv, Wo, bo):
    put_inputs, run, unpack = _get_runner()
    in_maps = _prep_inputs(X, Wq, bq, Wk, bk, Wv, bv, Wo, bo)
    dev_inputs = put_inputs(in_maps)
    outs = run(dev_inputs)
    res = unpack(outs)
    y = np.concatenate(
        [r["YT"].transpose(0, 2, 1) for r in res], axis=0)
    return np.ascontiguousarray(y.astype(np.float32))



# revision 27
# speedup vs baseline: 7.5832x; 7.5832x over previous
"""
Multi-head attention Trainium2 Bass kernel (B=16, S=1024, D=768, H=12, Dh=64).

Sharding: data parallel over batch — 8 cores x 2 batches each. Weights are
replicated; no collectives.

Per-core device algorithm (all matmuls bf16 with fp32 PSUM accumulation):
  1. QK^T projection: per head-pair tiles [Q^T_h0; Q^T_h1] and [K^T_h0; K^T_h1]
     of shape [128, S] (partition = head-dim e, stacked 2 heads), computed as
     lhsT = [W_h0 | W_h1] (stationary), rhs = X^T.  bq added on the PSUM->SBUF
     copy (per-partition scalar); bk is skipped entirely (constant-per-row
     terms cancel in softmax).
  2. V projection in [t, e] layout with a zero column per head that is later
     memset to 1 (V' = [V_h | 1]) -> AV matmul also produces softmax row-sums.
  3. scores^T tiles [t, s] via row-tiled (tile_position) pairs of K=64 matmuls
     (2 heads concurrently in the 128x128 array).  Q is pre-scaled by 1/256 so
     the scores PSUM holds u = raw/256; softmax runs without max subtraction
     (u in ~[-0.2, 0.2], exp(32u) is safe in fp32): ACT exp (scale=32) fused
     with the PSUM->SBUF copy.  In the foreign-starved last phase a subset of
     t-tiles run a 2-op custom DVE exp instead (dve_k).
  4. AV: O'^T[e|rowsum, s] = V'^T A^T accumulated over t tiles, emitted with a
     2-iteration lag so the PE FIFO never blocks on ACT or on the previous
     pair-chunk's normalize.
  5. normalize (async, off the PE critical path): one DVE copy evacuates
     O'|rowsum from PSUM (releases the accumulator in ~1.3us); rowsum rows
     are DMA-packed 4 to a tile so one DVE reciprocal serves 2 pair-chunks
     (reciprocal cost is free-size-driven); recip rows are DMA-unpacked to
     partition-0 seeds for the gpsimd broadcast, then multiplied into msa^T.
     Engines only touch base-partition-0 APs here (non-32-multiple bases are
     rejected and 32-multiple non-zero bases mis-execute on HW); DMA does
     the cross-partition moves.  Custom DVE ops (reciprocal_approx_fast,
     EXP32 offload) produce garbage on this deployment's hardware ucode and
     are not used.
  6. out-projection Y^T = Wo^T msa^T + bo' where bo' = bo + bv_flat @ Wo
     (folded on host), written to DRAM as Y^T and transposed on host.

Scheduling: one continuous foreign-work stream feeds the PE during the
ACT-gated attention microloop.  All projections (QKV of the next batch, the
output projection of the previous batch / previous chunk) are emitted as
single-PSUM-bank half-groups, double-buffered through a 2-buffer PSUM pool so
group->evacuate chains never stall the PE.  Attention is chunk-outer /
pair-inner so each batch's output projection overlaps chunk-wise; batch 0's
attention starts as soon as k0/q0(chunk0) are projected, with the rest of the
QKV stream pulled as foreign work.  This keeps the PE dense end-to-end (no
>3.4us idle windows, so the HAM clock stays at 2.4 GHz).
"""

import sys

for p in ("/opt/trn_rl_repo", "/root/.axon_site/_ro/trn_rl_repo"):
    if p not in sys.path:
        sys.path.insert(0, p)

import numpy as np
import ml_dtypes

B, S, D, H, Dh = 16, 1024, 768, 12, 64
NCORE = 8
BLOC = B // NCORE          # 2 batches per core
PAIRS = H // 2             # 6 head pairs
DT = D // 128              # 6 d-tiles (contraction tiles)
TT = S // 128              # 8 t-tiles
SC = S // 512              # 2 s-chunks
VW = H * (Dh + 1)          # 780: V' width incl. ones columns

DVE_K_DEFAULT = 0          # custom-DVE exp offload disabled (broken on HW)
POUR_WET = 3               # foreign matmuls pulled per t-iter (supply-rich)
POUR_DRY = 2               # same, in the foreign-starved last phase

_CACHE = {}

# ---- custom DVE exp (softmax exp offload from ACT to DVE) ------------------
# exp(32*u) = (1 + u + u^2/2 + u^3/6)^32 for |u| <= ~0.2 (scores pre-scaled
# by 1/256 so PSUM holds u).  Two DVE ops: cubic Horner, then 5 squarings.
_EXPA_CONSTS = {"s0": 1.0 / 6.0, "s1": 0.5, "imm2": 1.0}
_DVE_OPS_CACHE = {}


def _get_exp_ops():
    if "ops" in _DVE_OPS_CACHE:
        return _DVE_OPS_CACHE["ops"]
    from concourse.dve_spec import Spec, Src0, C0, C1, C2, sq, lower, _has_src1
    from concourse.dve_uop import DveOpSpec
    from concourse.dve_ops import (
        DveOp, OPS, _SUB_OPCODE_FOR_NAME, CUSTOM_DVE_SPECS)

    def make_op(name, spec, subdim=False):
        if name not in _SUB_OPCODE_FOR_NAME:
            _SUB_OPCODE_FOR_NAME[name] = 1 + len(OPS)
        shas = {}
        for ver in ("v3", "v4"):
            uops = lower(spec, ver=ver)
            shas[ver] = DveOpSpec(
                name=name, opcode=_SUB_OPCODE_FOR_NAME[name],
                uops=uops, rd1_en=_has_src1(spec)).sha(ver)
        op = DveOp(name, spec, subdim=subdim, uops_sha=shas)
        if all(o.name != name for o in OPS):
            OPS.append(op)
        CUSTOM_DVE_SPECS[name] = spec
        return op

    poly = make_op(
        "EXP32_POLY_ANT",
        Spec(body=((Src0 * C0 + C1) * Src0 + C2) * Src0 + C2,
             reference=lambda in0, in1, c0, c1, c2:
             ((in0 * c0 + c1) * in0 + c2) * in0 + c2))
    sq5 = make_op(
        "EXP32_SQ5_ANT",
        Spec(body=sq(sq(sq(sq(sq(Src0))))),
             reference=lambda in0, in1, c0, c1, c2:
             ((((in0 * in0) ** 2) ** 2) ** 2) ** 2))
    _DVE_OPS_CACHE["ops"] = (poly, sq5)
    return poly, sq5


def _emit_exp32(nc, out_bf, w_f32, psum_in):
    poly, sq5 = _get_exp_ops()
    c = _EXPA_CONSTS
    nc.vector._custom_dve(poly, out=w_f32, in0=psum_in,
                          s0=c["s0"], s1=c["s1"], imm2=c["imm2"])
    nc.vector._custom_dve(sq5, out=out_bf, in0=w_f32)


_done = object()


class _Stream:
    """FIFO of generators emitting foreign (non-attention) PE work.

    Generators yield None after each emitted matmul and yield hashable marks
    after notable evacuations.  ensure(mark) advances emission until the mark
    has been seen; pull(n) emits up to n foreign matmuls."""

    def __init__(self):
        self.gens = []
        self.marks = set()

    def add(self, gen):
        self.gens.append(gen)

    def pull(self, n):
        cnt = 0
        while self.gens and cnt < n:
            item = next(self.gens[0], _done)
            if item is _done:
                self.gens.pop(0)
            elif item is None:
                cnt += 1
            else:
                self.marks.add(item)
        return cnt

    def ensure(self, mark):
        while mark not in self.marks:
            if not self.gens:
                raise RuntimeError(f"stream exhausted before {mark}")
            item = next(self.gens[0], _done)
            if item is _done:
                self.gens.pop(0)
            elif item is not None:
                self.marks.add(item)

    def drain(self):
        while self.pull(1 << 20):
            pass


def _build_program(repeats=1, ablate=None, dve_k=DVE_K_DEFAULT,
                   recip_fast=True):
    import contextlib
    import concourse.tile as tile
    from concourse import bacc, mybir

    bf = mybir.dt.bfloat16
    f32 = mybir.dt.float32
    EXP = mybir.ActivationFunctionType.Exp

    nc = bacc.Bacc("TRN2", target_bir_lowering=False, debug=False,
                   num_devices=NCORE)

    XT = nc.dram_tensor("XT", [BLOC, D, S], bf, kind="ExternalInput").ap()
    WQ = nc.dram_tensor("WQ", [D, D], bf, kind="ExternalInput").ap()
    WK = nc.dram_tensor("WK", [D, D], bf, kind="ExternalInput").ap()
    WV = nc.dram_tensor("WV", [D, VW], bf, kind="ExternalInput").ap()
    WO = nc.dram_tensor("WO", [D, D], bf, kind="ExternalInput").ap()
    BQ = nc.dram_tensor("BQ", [128, PAIRS], f32, kind="ExternalInput").ap()
    BO = nc.dram_tensor("BO", [128, DT], f32, kind="ExternalInput").ap()
    YT = nc.dram_tensor("YT", [BLOC, D, S], f32, kind="ExternalOutput").ap()

    VB = BLOC * repeats  # virtual batches (b = vb % BLOC)
    dve_ts = {0: (), 1: (4,), 2: (2, 6), 3: (1, 4, 6),
              4: (1, 3, 5, 7)}[dve_k]

    with tile.TileContext(nc) as tc:
        with contextlib.ExitStack() as ctx:
            consts = ctx.enter_context(tc.tile_pool(name="consts", bufs=1))
            xt_p = ctx.enter_context(tc.tile_pool(name="xt", bufs=2 * DT))
            qk_p = ctx.enter_context(tc.tile_pool(name="qk", bufs=4 * PAIRS))
            vp_p = ctx.enter_context(tc.tile_pool(name="vp", bufs=2 * TT))
            a_p = ctx.enter_context(tc.tile_pool(name="a", bufs=4))
            if dve_ts:
                aw_p = ctx.enter_context(tc.tile_pool(name="aw", bufs=2))
            msa_p = ctx.enter_context(tc.tile_pool(name="msa", bufs=2 * DT))
            y_p = ctx.enter_context(tc.tile_pool(name="y", bufs=3))
            # normalize staging: unnormalized O' evacuated from PSUM (f32),
            # row-sums staged to partition-0 tiles, DMA-packed 4-per-tile so
            # one DVE reciprocal serves 2 pair-chunks (reciprocal cost is
            # free-size-driven: [4,512] costs the same ~3.3us as [1,512]),
            # then DMA-unpacked to partition-0 seeds for the gpsimd
            # broadcast.  Engines only ever touch base-partition-0 APs here
            # (non-zero base partitions mis-execute); DMA moves across
            # partitions.
            ou_p = ctx.enter_context(tc.tile_pool(name="ou", bufs=6))
            rs_p = ctx.enter_context(tc.tile_pool(name="rs", bufs=2))
            ri_p = ctx.enter_context(tc.tile_pool(name="ri", bufs=2))
            sd_p = ctx.enter_context(tc.tile_pool(name="sd", bufs=3))
            rb_p = ctx.enter_context(tc.tile_pool(name="rb", bufs=3))
            # PSUM (8 banks): scores 2x[128,1024]=4, foreign 2x[128,512]=2,
            # AV accumulators 2x[65,512]=2
            ps_sc = ctx.enter_context(
                tc.tile_pool(name="ps_sc", bufs=2, space="PSUM"))
            ps_fg = ctx.enter_context(
                tc.tile_pool(name="ps_fg", bufs=2, space="PSUM"))
            ps_av = ctx.enter_context(
                tc.tile_pool(name="ps_av", bufs=2, space="PSUM"))

            # ---- resident weights / biases -------------------------------
            # DMA order: (WQ[d], X[d]) interleaved so the first projection's
            # d=0 matmul starts after just two transfers; then WK (needed by
            # k0), WV (v tiles), WQ/WK rest is covered, WO + BO last (only
            # needed by the first output projection, ~100us in).
            wq_sb = []
            wk_sb = []
            wv_sb = []
            wo_sb = []

            xts = {}

            def load_xt(vb):
                b = vb % BLOC
                xs = []
                for d in range(DT):
                    t = xt_p.tile([128, S], bf, tag="xt", name=f"xt{d}")
                    nc.sync.dma_start(
                        out=t, in_=XT[b, d * 128:(d + 1) * 128, :])
                    xs.append(t)
                return xs

            bq_sb = consts.tile([128, PAIRS], f32, tag="bq")
            nc.sync.dma_start(out=bq_sb, in_=BQ)
            xts[0] = []
            for d in range(DT):
                t = consts.tile([128, D], bf, tag=f"wq{d}")
                nc.sync.dma_start(out=t, in_=WQ[d * 128:(d + 1) * 128, :])
                wq_sb.append(t)
                t = xt_p.tile([128, S], bf, tag="xt", name=f"xt{d}")
                nc.sync.dma_start(out=t, in_=XT[0, d * 128:(d + 1) * 128, :])
                xts[0].append(t)
            for d in range(DT):
                t = consts.tile([128, D], bf, tag=f"wk{d}")
                nc.sync.dma_start(out=t, in_=WK[d * 128:(d + 1) * 128, :])
                wk_sb.append(t)
            for d in range(DT):
                t = consts.tile([128, VW], bf, tag=f"wv{d}")
                nc.sync.dma_start(out=t, in_=WV[d * 128:(d + 1) * 128, :])
                wv_sb.append(t)
            for d in range(DT):
                t = consts.tile([128, D], bf, tag=f"wo{d}")
                nc.sync.dma_start(out=t, in_=WO[d * 128:(d + 1) * 128, :])
                wo_sb.append(t)
            bo_sb = consts.tile([128, DT], f32, tag="bo")
            nc.sync.dma_start(out=bo_sb, in_=BO)

            qts = {}
            kts = {}
            vps = {}
            msas = {}

            def qkv_stream(vb):
                """Foreign generator: QKV projection of batch vb as
                single-bank half-groups.  Emission order lets attention(vb)
                start after k0 + q0(chunk0): k0, q0c0, v0..v7, then the
                remaining pairs' k/q(c0), then all q(c1) halves."""
                xt_sb = xts[vb]
                qt, kt, vp = qts[vb], kts[vb], vps[vb]

                def emit_qk(kind, p, c):
                    w = wq_sb if kind == "q" else wk_sb
                    ps = ps_fg.tile([128, 512], f32, tag="fg", name="ps_qk")
                    for d in range(DT):
                        nc.tensor.matmul(
                            ps,
                            lhsT=w[d][:, p * 128:(p + 1) * 128],
                            rhs=xt_sb[d][:, c * 512:(c + 1) * 512],
                            start=(d == 0), stop=(d == DT - 1))
                        yield None
                    dst = (qt if kind == "q" else kt)[p]
                    if kind == "q":
                        # Q scaled by 1/256 so scores PSUM holds u = raw/256
                        nc.vector.tensor_scalar(
                            dst[:, c * 512:(c + 1) * 512], ps,
                            1.0 / 256.0, bq_sb[:, p:p + 1],
                            mybir.AluOpType.mult, mybir.AluOpType.add)
                    else:
                        nc.vector.tensor_copy(
                            dst[:, c * 512:(c + 1) * 512], ps)

                def emit_pair(kind, p):
                    lst = qt if kind == "q" else kt
                    while len(lst) <= p:
                        lst.append(qk_p.tile([128, S], bf, tag="qk",
                                             name=f"{kind}{len(lst)}"))
                    yield from emit_qk(kind, p, 0)
                    if kind == "k":
                        yield from emit_qk(kind, p, 1)
                        yield ("k", vb, p)
                    else:
                        yield ("q", vb, p, 0)

                def emit_v(T):
                    vt = vp_p.tile([128, VW], bf, tag="vp", name="vp")
                    ps = ps_fg.tile([128, 512], f32, tag="fg", name="ps_vA")
                    for d in range(DT):
                        nc.tensor.matmul(
                            ps,
                            lhsT=xt_sb[d][:, T * 128:(T + 1) * 128],
                            rhs=wv_sb[d][:, 0:512],
                            start=(d == 0), stop=(d == DT - 1))
                        yield None
                    nc.vector.tensor_copy(vt[:, 0:512], ps)
                    ps = ps_fg.tile([128, 512], f32, tag="fg", name="ps_vB")
                    for d in range(DT):
                        nc.tensor.matmul(
                            ps[:, 0:VW - 512],
                            lhsT=xt_sb[d][:, T * 128:(T + 1) * 128],
                            rhs=wv_sb[d][:, 512:VW],
                            start=(d == 0), stop=(d == DT - 1))
                        yield None
                    nc.vector.tensor_copy(vt[:, 512:VW], ps[:, 0:VW - 512])
                    v3 = vt.rearrange("p (h e) -> p h e", e=Dh + 1)
                    nc.vector.memset(v3[:, :, Dh:Dh + 1], 1.0)
                    vp.append(vt)
                    yield ("v", vb, T)

                # q0 (both chunks) first: q only needs WQ + X, so on the
                # first batch this covers the DMA window until WK/WV land
                yield from emit_pair("q", 0)
                yield from emit_qk("q", 0, 1)
                yield ("q", vb, 0, 1)
                yield from emit_pair("k", 0)
                for T in range(TT):
                    yield from emit_v(T)
                for p in range(1, PAIRS):
                    yield from emit_pair("k", p)
                    yield from emit_pair("q", p)
                for p in range(1, PAIRS):
                    yield from emit_qk("q", p, 1)
                    yield ("q", vb, p, 1)

            def proj_stream(vb, c):
                """Foreign generator: output projection of batch vb, query
                chunk c (only needs msa columns of that chunk)."""
                b = vb % BLOC
                msa_sb = msas[vb]
                for o in range(DT):
                    ps = ps_fg.tile([128, 512], f32, tag="fg", name="ps_o")
                    for d in range(DT):
                        nc.tensor.matmul(
                            ps,
                            lhsT=wo_sb[d][:, o * 128:(o + 1) * 128],
                            rhs=msa_sb[d][:, c * 512:(c + 1) * 512],
                            start=(d == 0), stop=(d == DT - 1))
                        yield None
                    y = y_p.tile([128, 512], f32, tag="y", name="y")
                    nc.vector.tensor_scalar_add(y, ps, bo_sb[:, o:o + 1])
                    if ablate != "no_out_dma":
                        nc.sync.dma_start(
                            out=YT[b, o * 128:(o + 1) * 128,
                                   c * 512:(c + 1) * 512],
                            in_=y)

            def attention(vb, stream, dts, pour, tail=False):
                qt, kt, vp = qts[vb], kts[vb], vps[vb]
                msa_sb = msas[vb]
                for p in range(PAIRS):
                    msa_sb.append(msa_p.tile([128, S], bf, tag="msa",
                                             name=f"msa{p}"))
                for c in range(SC):
                    if c == 1:
                        stream.add(proj_stream(vb, 0))
                    # normalize batches of 2 pair-chunks; in the very last
                    # chunk the final pairs get their own batch so the tail
                    # chain (recip -> broadcast -> mul -> out-projection) is
                    # as short as possible
                    groups = [(0, 1), (2, 3), (4,), (5,)] \
                        if tail and c == SC - 1 else [(0, 1), (2, 3), (4, 5)]
                    gmap = {}
                    for g in groups:
                        for gi, p_ in enumerate(g):
                            gmap[p_] = (g, gi)
                    for p in range(PAIRS):
                        g, gi = gmap[p]
                        stream.ensure(("k", vb, p))
                        stream.ensure(("q", vb, p, c))
                        if gi == 0:
                            rs = rs_p.tile([4, 512], f32, tag="rs",
                                           name="rs")
                            entries = []
                        po = [ps_av.tile([65, 512], f32, tag="av",
                                         name=f"po{h}")
                              for h in range(2)]

                        def emit_av(T, at):
                            for h in range(2):
                                nc.tensor.matmul(
                                    po[h],
                                    lhsT=vp[T][
                                        :, (2 * p + h) * (Dh + 1):
                                        (2 * p + h + 1) * (Dh + 1)],
                                    rhs=at[:, h * 512:(h + 1) * 512],
                                    start=(T == 0), stop=(T == TT - 1))

                        # AV runs two t-tiles behind exp so the PE FIFO never
                        # blocks on ACT nor on the previous pair-chunk's po
                        # release (normalize is async but takes ~2.5us).
                        pend = []
                        for T in range(TT):
                            if c == 0:
                                stream.ensure(("v", vb, T))
                            pss = ps_sc.tile([128, 1024], f32, tag="sc",
                                             name="pss")
                            for h in range(2):
                                nc.tensor.matmul(
                                    pss[:, h * 512:(h + 1) * 512],
                                    lhsT=kt[p][h * 64:(h + 1) * 64,
                                               T * 128:(T + 1) * 128],
                                    rhs=qt[p][h * 64:(h + 1) * 64,
                                              c * 512:(c + 1) * 512],
                                    start=True, stop=True,
                                    tile_position=(h * 64, 0))
                            at = a_p.tile([128, 1024], bf, tag="a", name="at")
                            if T in dts:
                                aw = aw_p.tile([128, 1024], f32, tag="aw",
                                               name="aw")
                                _emit_exp32(nc, at, aw, pss)
                            else:
                                nc.scalar.activation(at, pss, EXP, scale=32.0)
                            stream.pull(pour)
                            if len(pend) >= 2:
                                emit_av(*pend.pop(0))
                            pend.append((T, at))
                        for e in pend:
                            emit_av(*e)
                        # evacuate O' (bf16) and pack the rowsum row; PSUM
                        # accumulators are released after ~1.2us so the next
                        # pair-chunk's AV never stalls.
                        for h in range(2):
                            # one copy evacuates O' AND the rowsum row;
                            # releases the PSUM accumulator in ~1.3us
                            ou = ou_p.tile([65, 512], f32, tag="ou",
                                           name="ou")
                            nc.vector.tensor_copy(ou, po[h])
                            j = 2 * gi + h
                            nc.sync.dma_start(out=rs[j:j + 1, :],
                                              in_=ou[64:65, :])
                            entries.append((p, h, ou))
                        if p == g[-1]:
                            # batched normalize (async): one reciprocal
                            # serves the whole group's rowsums
                            n = 2 * len(g)
                            ri = ri_p.tile([4, 512], f32, tag="ri",
                                           name="ri")
                            nc.vector.reciprocal(ri[0:n, :], rs[0:n, :])
                            for j, (pp, h, ou) in enumerate(entries):
                                sd = sd_p.tile([1, 512], f32, tag="sd",
                                               name="sd")
                                nc.sync.dma_start(out=sd,
                                                  in_=ri[j:j + 1, :])
                                rb = rb_p.tile([64, 512], f32, tag="rb",
                                               name="rb")
                                nc.gpsimd.partition_broadcast(rb, sd)
                                nc.vector.tensor_mul(
                                    msa_sb[pp][h * 64:(h + 1) * 64,
                                               c * 512:(c + 1) * 512],
                                    ou[0:64, :], rb)
                                stream.pull(1)
                        stream.pull(pour)

            # ---- pipelined schedule over virtual batches ------------------
            stream = _Stream()
            qts[0], kts[0], vps[0] = [], [], []
            stream.add(qkv_stream(0))
            # ACT exp-table load (~2.7us) off the critical path: a dummy exp
            # during the QKV phase triggers PSEUDO_LOAD_ACT_FUNC_SET early.
            warm = consts.tile([1, 2], f32, tag="warm")
            nc.vector.memset(warm, 0.0)
            nc.scalar.activation(warm, warm, EXP)

            for vb in range(VB):
                if vb >= 1:
                    stream.add(proj_stream(vb - 1, 1))
                if vb + 1 < VB:
                    xts[vb + 1] = load_xt(vb + 1)
                    qts[vb + 1], kts[vb + 1], vps[vb + 1] = [], [], []
                    stream.add(qkv_stream(vb + 1))
                msas[vb] = []
                last = vb + 1 >= VB
                attention(vb, stream,
                          dts=(dve_ts if last else ()),
                          pour=(POUR_DRY if last else POUR_WET),
                          tail=last)
                for dd in (qts, kts, vps, xts):
                    dd.pop(vb - 1, None)
            stream.add(proj_stream(VB - 1, 1))
            stream.drain()

    nc.compile()
    return nc


def _prep_inputs(X, Wq, bq, Wk, bk, Wv, bv, Wo, bo):
    bf16 = ml_dtypes.bfloat16
    X = np.asarray(X, dtype=np.float32)
    # per-core X^T: [core][BLOC, D, S]
    xt = np.ascontiguousarray(
        X.reshape(NCORE, BLOC, S, D).transpose(0, 1, 3, 2)).astype(bf16)
    wq = np.ascontiguousarray(
        np.asarray(Wq, np.float32).transpose(1, 0, 2).reshape(D, D)).astype(bf16)
    wk = np.ascontiguousarray(
        np.asarray(Wk, np.float32).transpose(1, 0, 2).reshape(D, D)).astype(bf16)
    wv = np.zeros((D, VW), np.float32)
    Wv = np.asarray(Wv, np.float32)
    for h in range(H):
        wv[:, h * (Dh + 1):h * (Dh + 1) + Dh] = Wv[h]
    wv = wv.astype(bf16)
    wo = np.asarray(Wo, np.float32).astype(bf16)
    bq2 = np.ascontiguousarray(
        np.asarray(bq, np.float32).reshape(PAIRS, 128).T) / 256.0
    bo_eff = np.asarray(bo, np.float32) + \
        np.asarray(bv, np.float32).reshape(-1) @ np.asarray(Wo, np.float32)
    bo2 = np.ascontiguousarray(bo_eff.reshape(DT, 128).T.astype(np.float32))
    in_maps = [
        {"XT": xt[c], "WQ": wq, "WK": wk, "WV": wv, "WO": wo,
         "BQ": bq2, "BO": bo2}
        for c in range(NCORE)
    ]
    return in_maps


def _get_runner(repeats=1, ablate=None, dve_k=DVE_K_DEFAULT):
    """Build (once) a jitted SPMD runner over the 8 cores, modeled on
    bass2jax.run_bass_via_pjrt but cached so repeat calls don't re-trace."""
    key = ("runner", repeats, ablate, dve_k)
    if key in _CACHE:
        return _CACHE[key]

    import jax
    import numpy as _np
    from jax.sharding import Mesh, PartitionSpec, NamedSharding
    from jax.experimental.shard_map import shard_map
    from concourse.bass2jax import (
        _bass_exec_p, install_neuronx_cc_hook, partition_id_tensor)

    nc = _build_program(repeats=repeats, ablate=ablate, dve_k=dve_k)
    install_neuronx_cc_hook()

    import concourse.mybir as _mybir
    in_names, out_names, out_avals, zero_shapes = [], [], [], []
    partition_name = (nc.partition_id_tensor.name
                      if nc.partition_id_tensor else None)
    for alloc in nc.m.functions[0].allocations:
        if not isinstance(alloc, _mybir.MemoryLocationSet):
            continue
        name = alloc.memorylocations[0].name
        if alloc.kind == "ExternalInput":
            if name != partition_name:
                in_names.append(name)
        elif alloc.kind == "ExternalOutput":
            shape = tuple(alloc.tensor_shape)
            dtype = _mybir.dt.np(alloc.dtype)
            out_names.append(name)
            out_avals.append(jax.core.ShapedArray(shape, dtype))
            zero_shapes.append((shape, dtype))
    n_params = len(in_names)
    n_outs = len(out_names)
    all_in_names = in_names + out_names
    if partition_name is not None:
        all_in_names = all_in_names + [partition_name]

    def _body(*args):
        operands = list(args)
        if partition_name is not None:
            operands.append(partition_id_tensor())
        outs = _bass_exec_p.bind(
            *operands,
            out_avals=tuple(out_avals),
            in_names=tuple(all_in_names),
            out_names=tuple(out_names),
            lowering_input_output_aliases=(),
            sim_require_finite=True,
            sim_require_nnan=True,
            nc=nc,
        )
        return tuple(outs)

    devices = jax.devices()[:NCORE]
    mesh = Mesh(_np.asarray(devices), ("core",))
    in_specs = (PartitionSpec("core"),) * (n_params + n_outs)
    out_specs = (PartitionSpec("core"),) * n_outs
    # NOTE: no donation — the kernel writes every output element, so the
    # custom call's self-allocated (uninit) output buffers are fine, and the
    # zero "output operand" arrays can be created once and reused across
    # calls instead of being shipped host->device (50 MB) per call.
    sharded = jax.jit(
        shard_map(_body, mesh=mesh, in_specs=in_specs, out_specs=out_specs,
                  check_rep=False),
        keep_unused=True)
    shard = NamedSharding(mesh, PartitionSpec("core"))
    zeros_dev = [
        jax.device_put(_np.zeros((NCORE * s[0], *s[1:]), d), shard)
        for s, d in zero_shapes
    ]

    def put_inputs(in_maps):
        # concatenate along axis 0 (per-core stacking)
        concat = []
        for nm in in_names:
            arrs = [_np.asarray(in_maps[c][nm]) for c in range(NCORE)]
            concat.append(_np.concatenate(arrs, axis=0))
        return [jax.device_put(a, shard) for a in concat]

    _CACHE[("sharded", repeats, ablate, dve_k)] = (sharded, zeros_dev)

    def run(dev_inputs):
        outs = sharded(*dev_inputs, *zeros_dev)
        jax.block_until_ready(outs)
        return outs

    def unpack(outs):
        res = []
        for c in range(NCORE):
            d = {}
            for i, nm in enumerate(out_names):
                full = _np.asarray(outs[i])
                d[nm] = full.reshape(NCORE, *out_avals[i].shape)[c]
            res.append(d)
        return res

    _CACHE[key] = (put_inputs, run, unpack)
    return _CACHE[key]


def kernel(X, Wq, bq, Wk, bk, Wv, bv, Wo, bo):
    put_inputs, run, unpack = _get_runner()
    in_maps = _prep_inputs(X, Wq, bq, Wk, bk, Wv, bv, Wo, bo)
    dev_inputs = put_inputs(in_maps)
    outs = run(dev_inputs)
    res = unpack(outs)
    y = np.concatenate(
        [r["YT"].transpose(0, 2, 1) for r in res], axis=0)
    return np.ascontiguousarray(y.astype(np.float32))
